# revision 2
# baseline (speedup 1.0000x reference)
"""Trainium2 Bass kernel for nn_Affine_Linear_Abla_Quat — v2.

Math (per batch b, point n, channel d), g = third column of R(q):
    Y_i = A.x_i + B.(g x x)_i + (C-A).(g_i (g.x))
    g = (xz+yw, yz-xw, (zz+ww-xx-yy)/2) / h,  h = (xx+yy+zz+ww)/2

v2 changes vs v1:
  * DVE work fused into 11 instructions/segment (was 19): the four
    quaternion cross-terms as one outer-product instr (x,y)x(z,w), the
    q-pair sums as one 2-slot instr, and all nine g_a*x_b products as a
    single 3x3 outer instr whose diagonal feeds the dot and whose
    off-diagonals feed the +/- cross matmuls directly.  J slots are sent
    as [A=(x..y), B=(z..w)] so both the square-pairs and the product
    pairs are plain slices (flat order x,z,y,w).
  * per-segment tiles keep (d_half, n) flattened into one packed axis so
    every compute AP stays within the TENSOR3D 3-free-dim limit; the
    d_half split reappears only as matmul slice offsets.
  * first/last supersteps split in half (256 pts) to shrink the DMA/ACT
    pipeline ramp at the head and the PE/evac drain at the tail.
  * PE piece order A, B, -B, C-A: the ct-dependent block runs last.
  * output DMA on the (otherwise idle) Pool ring: Pool engine compute is
    poison (it halves DVE throughput via shared SBUF ports) but its DGE
    dispatch is free.
Sharding: data-parallel over batch B=8 -> one batch per NeuronCore.
"""

import numpy as np

import concourse.bass as bass
import concourse.tile as tile
from concourse import mybir
from concourse.bass_utils import run_bass_kernel_spmd

F16 = mybir.dt.float16
F32 = mybir.dt.float32

N_CORES = 8
NPTS = 4096          # points per core (batch dim sharded)
D = 256              # in channels
F = 256              # out channels
P = 128              # partitions
SN = 512             # points per full super-step
NS = NPTS // SN
HN = 2 * SN          # (d_half, n) packed axis
RSQRT2 = 0.7071067811865476

# (superstep, offset, count): half-size segments at head and tail
SEGS = ([(0, 0, 256), (0, 256, 256)]
        + [(s, 0, 512) for s in range(1, NS - 1)]
        + [(NS - 1, 0, 256), (NS - 1, 256, 256)])

ADD = mybir.AluOpType.add
SUB = mybir.AluOpType.subtract
MUL = mybir.AluOpType.mult


def _act_raw(nc, out, in_, func, scale=1.0):
    """InstActivation without the wrapper's Reciprocal guard (fp16 path)."""
    eng = nc.scalar
    ins = [eng.lower_ap(in_),
           mybir.ImmediateValue(dtype=mybir.dt.float32, value=0.0),
           mybir.ImmediateValue(dtype=mybir.dt.float32, value=scale),
           mybir.ImmediateValue(dtype=mybir.dt.float32, value=0.0)]
    return eng.add_instruction(mybir.InstActivation(
        name=nc.get_next_instruction_name(), func=func,
        ins=ins, outs=[eng.lower_ap(out)]))


def _split_multi_waits(nc):
    """This container's walrus rejects instructions carrying more than one
    sync wait. Hoist extra waits onto same-engine NoOps inserted directly
    before the offending instruction."""
    ctr = 0
    for f in nc.m.functions:
        for bb in f.blocks:
            out = []
            for inst in bb.instructions:
                si = inst.sync_info
                if si is not None and si.on_wait and len(si.on_wait) > 1:
                    waits = list(si.on_wait)
                    for w in waits[:-1]:
                        nop = mybir.InstNoOp(
                            name=f"waitnop_{ctr}", ins=[], outs=[])
                        ctr += 1
                        nop.engine = inst.engine
                        nop.bass_nofuse = True
                        nop.sync_info = mybir.SyncInfo(
                            on_wait=[w], on_update=[])
                        out.append(nop)
                    si.on_wait.clear()
                    si.on_wait.append(waits[-1])
                out.append(inst)
            bb.instructions[:] = out


def _dedupe_ldweights(nc):
    """Drop PE Ldweights reloads of an already-resident stationary (Tile
    emits one per matmul; the reload forces a full array drain)."""
    ctr = 0
    for f in nc.m.functions:
        for bb in f.blocks:
            out = []
            last_w = None
            for inst in bb.instructions:
                if inst.engine != mybir.EngineType.PE:
                    out.append(inst)
                    continue
                if inst.opcode == "Ldweights":
                    sig = str(inst.ins[0])
                    si = inst.sync_info
                    has_upd = bool(si is not None and si.on_update)
                    if sig == last_w and not has_upd:
                        if si is not None and si.on_wait:
                            nop = mybir.InstNoOp(
                                name=f"ldwnop_{ctr}", ins=[], outs=[])
                            ctr += 1
                            nop.engine = mybir.EngineType.PE
                            nop.bass_nofuse = True
                            nop.sync_info = mybir.SyncInfo(
                                on_wait=list(si.on_wait), on_update=[])
                            out.append(nop)
                        continue
                    last_w = sig
                elif inst.opcode != "Matmult":
                    last_w = None
                out.append(inst)
            bb.instructions[:] = out


def build_kernel(fixup=True):
    nc = bass.Bass("TRN2", target_bir_lowering=False, debug=False)
    # inputs, already fp16 + device-layout on host (pure relayout/cast)
    # J slots as [A, B, h, n] with A-major flat order (x, z, y, w):
    # jt[:, :, 0] = (x, y), jt[:, :, 1] = (z, w)
    j_d = nc.dram_tensor("JB", [NS, P, 2, 2, 2, SN], F16,
                         kind="ExternalInput").ap()
    x_d = nc.dram_tensor("XB", [NS, P, 3, 2, SN], F16,
                         kind="ExternalInput").ap()
    wa_d = nc.dram_tensor("WA", [P, 2, F], F16, kind="ExternalInput").ap()
    wb_d = nc.dram_tensor("WB", [P, 2, F], F16, kind="ExternalInput").ap()
    wbn_d = nc.dram_tensor("WBN", [P, 2, F], F16, kind="ExternalInput").ap()
    wc_d = nc.dram_tensor("WC", [P, 2, F], F16, kind="ExternalInput").ap()
    y_d = nc.dram_tensor("YB", [NS, P, 2, 3, SN], F16,
                         kind="ExternalOutput").ap()

    with tile.TileContext(nc) as tc:
        _body(nc, tc, j_d, x_d, (wa_d, wb_d, wbn_d, wc_d), y_d)
    from concourse import bass as _b
    _b._bass_rust.move_matmul_waits_to_ldweights(nc.m)
    _dedupe_ldweights(nc)
    if fixup:
        _split_multi_waits(nc)
    return nc


def _body(nc, tc, j_d, x_d, w_d, y_d):
    from contextlib import ExitStack
    ctx = ExitStack()
    with ctx:
        singles = ctx.enter_context(tc.tile_pool(name="singles", bufs=1))
        inp = ctx.enter_context(tc.tile_pool(name="inp", bufs=2))
        mid = ctx.enter_context(tc.tile_pool(name="mid", bufs=2))
        small = ctx.enter_context(tc.tile_pool(name="small", bufs=3))
        ysbp = ctx.enter_context(tc.tile_pool(name="ysb", bufs=2))
        ypps = ctx.enter_context(tc.tile_pool(name="ypps", bufs=1,
                                              space="PSUM"))

        # preload the reciprocal_and_small ACT table set while the first
        # DMAs are in flight (Square/Copy/Reciprocal all live in this set)
        warm = singles.tile([P, 16], F16, tag="actwarm")
        nc.gpsimd.memset(warm[:], 1.0)
        _act_raw(nc, warm[:], warm[:],
                 mybir.ActivationFunctionType.Reciprocal)

        # first segment's loads go out before the (PE-only) weights
        jt0, xt0 = _load_seg(nc, inp, j_d, x_d, SEGS[0])

        wts = []
        for name, wd in zip(("wa", "wb", "wbn", "wc"), w_d):
            wt = singles.tile([P, 2, F], F16, tag=f"w_{name}", name=f"w{name}")
            nc.sync.dma_start(wt[:], wd)
            wts.append(wt)

        prev = None  # (yps, seg) pending evacuation
        loaded = (jt0, xt0)
        for gi, seg in enumerate(SEGS):
            jt, xt = loaded
            if gi + 1 < len(SEGS):
                loaded = _load_seg(nc, inp, j_d, x_d, SEGS[gi + 1])

            sn = seg[2]
            hn = 2 * sn
            # ---- ACT: half-squares; flat slots (xx, zz, yy, ww)/2 ----
            sq = mid.tile([P, 2, 2, HN], F16, tag="sq", name="sq")
            nc.scalar.activation(sq[:, :, :, :hn], jt[:, :, :, :hn],
                                 mybir.ActivationFunctionType.Square,
                                 scale=RSQRT2)

            # ---- DVE chain (11 instrs, 27 units) ----
            # P4[a,b] = (x,y)_a * (z,w)_b  ->  (xz, xw, yz, yw)
            p4 = mid.tile([P, 2, 2, HN], F16, tag="p4", name="p4")
            nc.vector.tensor_tensor(
                out=p4[:, :, :, :hn],
                in0=jt[:, 0, :, :hn].unsqueeze(2).broadcast_to([P, 2, 2, hn]),
                in1=jt[:, 1, :, :hn].unsqueeze(1).broadcast_to([P, 2, 2, hn]),
                op=MUL)
            # q12 = (xx+yy, zz+ww)/2
            q12 = small.tile([P, 2, HN], F16, tag="q12", name="q12")
            nc.vector.tensor_tensor(out=q12[:, :, :hn],
                                    in0=sq[:, :, 0, :hn],
                                    in1=sq[:, :, 1, :hn], op=ADD)
            sh = small.tile([P, HN], F16, tag="sh", name="sh")
            nc.vector.tensor_tensor(out=sh[:, :hn], in0=q12[:, 0, :hn],
                                    in1=q12[:, 1, :hn], op=ADD)
            # ACT: invh = 2/s (issued as early as possible)
            invh = small.tile([P, HN], F16, tag="invh", name="invh")
            _act_raw(nc, invh[:, :hn], sh[:, :hn],
                     mybir.ActivationFunctionType.Reciprocal)

            vt = mid.tile([P, 3, HN], F16, tag="vt", name="vt")
            nc.vector.tensor_tensor(out=vt[:, 2, :hn],
                                    in0=q12[:, 1, :hn],
                                    in1=q12[:, 0, :hn], op=SUB)
            nc.vector.tensor_tensor(out=vt[:, 0, :hn],
                                    in0=p4[:, 0, 0, :hn],
                                    in1=p4[:, 1, 1, :hn], op=ADD)
            nc.vector.tensor_tensor(out=vt[:, 1, :hn],
                                    in0=p4[:, 1, 0, :hn],
                                    in1=p4[:, 0, 1, :hn], op=SUB)
            g = mid.tile([P, 3, HN], F16, tag="g", name="g")
            nc.vector.tensor_tensor(
                out=g[:, :, :hn], in0=vt[:, :, :hn],
                in1=invh[:, :hn].unsqueeze(1).broadcast_to([P, 3, hn]),
                op=MUL)
            # O[a,b] = g_a * x_b : diag = pd, off-diag = cross pieces
            ot = mid.tile([P, 3, 3, HN], F16, tag="ot", name="ot")
            nc.vector.tensor_tensor(
                out=ot[:, :, :, :hn],
                in0=g[:, :, :hn].unsqueeze(2).broadcast_to([P, 3, 3, hn]),
                in1=xt[:, :, :hn].unsqueeze(1).broadcast_to([P, 3, 3, hn]),
                op=MUL)
            d1 = small.tile([P, HN], F16, tag="d1", name="d1")
            nc.vector.tensor_tensor(out=d1[:, :hn], in0=ot[:, 0, 0, :hn],
                                    in1=ot[:, 1, 1, :hn], op=ADD)
            dot = small.tile([P, HN], F16, tag="dot", name="dot")
            nc.vector.tensor_tensor(out=dot[:, :hn], in0=d1[:, :hn],
                                    in1=ot[:, 2, 2, :hn], op=ADD)
            ct = mid.tile([P, 3, HN], F16, tag="ct", name="ct")
            nc.vector.tensor_tensor(
                out=ct[:, :, :hn], in0=g[:, :, :hn],
                in1=dot[:, :hn].unsqueeze(1).broadcast_to([P, 3, hn]),
                op=MUL)

            # ---- PE: weights stationary, terms moving; ct-block last ----
            yp0 = ypps.tile([P, 3, SN], F32, tag="yp0", name="yp0")
            yp1 = ypps.tile([P, 3, SN], F32, tag="yp1", name="yp1")
            yps = (yp0, yp1)
            wa, wb, wbn, wc = wts
            crp = [(1, 2), (2, 0), (0, 1)]
            crn = [(2, 1), (0, 2), (1, 0)]
            pieces = [
                (wa, lambda i, h: xt[:, i, h * sn:(h + 1) * sn]),
                (wb, lambda i, h: ot[:, crp[i][0], crp[i][1],
                                     h * sn:(h + 1) * sn]),
                (wbn, lambda i, h: ot[:, crn[i][0], crn[i][1],
                                      h * sn:(h + 1) * sn]),
                (wc, lambda i, h: ct[:, i, h * sn:(h + 1) * sn]),
            ]
            for pi, (wt, term) in enumerate(pieces):
                for fh in range(2):
                    for h in range(2):
                        piece = wt[:, h, fh * P:(fh + 1) * P]
                        for i in range(3):
                            nc.tensor.matmul(
                                yps[fh][:, i, :sn],
                                lhsT=piece,
                                rhs=term(i, h),
                                start=(pi == 0 and h == 0),
                                stop=(pi == 3 and h == 1))

            # ---- evacuate the PREVIOUS segment's PSUM ----
            if prev is not None:
                _evac(nc, ysbp, prev, y_d)
            prev = (yps, seg)
        _evac(nc, ysbp, prev, y_d)


def _load_seg(nc, inp, j_d, x_d, seg):
    s, off, sn = seg
    hn = 2 * sn
    jt = inp.tile([P, 2, 2, HN], F16, tag="jt", name="jt")
    xt = inp.tile([P, 3, HN], F16, tag="xt", name="xt")
    if sn == SN:
        nc.sync.dma_start(jt[:, :, :, :hn],
                          j_d[s][:, :, :, :, off:off + sn])
        nc.sync.dma_start(xt[:, :, :hn], x_d[s][:, :, :, off:off + sn])
    else:
        for h in range(2):
            nc.sync.dma_start(jt[:, :, :, h * sn:(h + 1) * sn],
                              j_d[s][:, :, :, h, off:off + sn])
            nc.sync.dma_start(xt[:, :, h * sn:(h + 1) * sn],
                              x_d[s][:, :, h, off:off + sn])
    return jt, xt


def _evac(nc, ysbp, prev, y_d):
    (yp0, yp1), (s, off, sn) = prev
    ysb = ysbp.tile([P, 2, 3, SN], F16, tag="ysb", name="ysb")
    nc.scalar.copy(ysb[:, 0, :, :sn], yp0[:, :, :sn])
    nc.scalar.copy(ysb[:, 1, :, :sn], yp1[:, :, :sn])
    nc.gpsimd.dma_start(y_d[s][:, :, :, off:off + sn], ysb[:, :, :, :sn])


_BUILT = {}

# test-harness hooks (ignored in normal use)
TRACE = False
LAST_EXEC_NS = None
LAST_RESULT = None


def _get_nc():
    if "nc" not in _BUILT:
        _BUILT["nc"] = build_kernel()
    return _BUILT["nc"]


def _pack_inputs(X, J, A, B, C):
    def packw(M):
        return np.ascontiguousarray(
            M.T.astype(np.float16).reshape(2, P, F).transpose(1, 0, 2))

    wa, wb, wbn, wc = packw(A), packw(B), packw(-B), packw(C - A)

    in_maps = []
    for b in range(N_CORES):
        # [n, d, c] -> [s, p, c, h, nn]
        xb = X[b].astype(np.float16).reshape(NS, SN, 2, P, 3)
        xb = np.ascontiguousarray(xb.transpose(0, 3, 4, 2, 1))
        # J slots natural (x, y, z, w): A-pairs (x,y)/(z,w) feed the
        # product outer; B-columns (x,z)/(y,w)... (xx,zz)+(yy,ww) feed q12
        jb = J[b].astype(np.float16).reshape(NS, SN, 2, P, 4)
        jb = np.ascontiguousarray(jb.transpose(0, 3, 4, 2, 1)).reshape(
            NS, P, 2, 2, 2, SN)
        in_maps.append({
            "XB": xb, "JB": jb,
            "WA": wa, "WB": wb, "WBN": wbn, "WC": wc,
        })
    return in_maps


def kernel(X, J, A, B, C):
    """X [8,4096,256,3] f32, J [8,4096,256,4] f32, A/B/C [256,256] f32
    -> Y [8,4096,256,3] f32."""
    X = np.asarray(X)
    J = np.asarray(J)
    A = np.asarray(A, dtype=np.float32)
    B = np.asarray(B, dtype=np.float32)
    C = np.asarray(C, dtype=np.float32)

    nc = _get_nc()
    in_maps = _pack_inputs(X, J, A, B, C)
    global LAST_EXEC_NS, LAST_RESULT
    res = run_bass_kernel_spmd(nc, in_maps, core_ids=list(range(N_CORES)),
                               trace=TRACE)
    LAST_EXEC_NS = res.exec_time_ns
    LAST_RESULT = res
    # device YB [s, p(f_local), fh, i, nn] -> [n, f, i] fp32
    out = np.empty((N_CORES, NPTS, F, 3), dtype=np.float32)
    for b in range(N_CORES):
        yb = res.results[b]["YB"].reshape(NS, P, 2, 3, SN).astype(np.float32)
        out[b] = yb.transpose(0, 4, 2, 1, 3).reshape(NPTS, F, 3)
    return np.ascontiguousarray(out)


# revision 3
# speedup vs baseline: 1.0088x; 1.0088x over previous
"""Trainium2 Bass kernel for nn_Affine_Linear_Abla_Quat — v2.

Math (per batch b, point n, channel d), g = third column of R(q):
    Y_i = A.x_i + B.(g x x)_i + (C-A).(g_i (g.x))
    g = (xz+yw, yz-xw, (zz+ww-xx-yy)/2) / h,  h = (xx+yy+zz+ww)/2

v2 changes vs v1:
  * DVE work fused into 11 instructions/segment (was 19): the four
    quaternion cross-terms as one outer-product instr (x,y)x(z,w), the
    q-pair sums as one 2-slot instr, and all nine g_a*x_b products as a
    single 3x3 outer instr whose diagonal feeds the dot and whose
    off-diagonals feed the +/- cross matmuls directly.  J slots are sent
    as [A=(x..y), B=(z..w)] so both the square-pairs and the product
    pairs are plain slices (flat order x,z,y,w).
  * per-segment tiles keep (d_half, n) flattened into one packed axis so
    every compute AP stays within the TENSOR3D 3-free-dim limit; the
    d_half split reappears only as matmul slice offsets.
  * first/last supersteps split in half (256 pts) to shrink the DMA/ACT
    pipeline ramp at the head and the PE/evac drain at the tail.
  * PE piece order A, B, -B, C-A: the ct-dependent block runs last.
  * output DMA on the (otherwise idle) Pool ring: Pool engine compute is
    poison (it halves DVE throughput via shared SBUF ports) but its DGE
    dispatch is free.
Sharding: data-parallel over batch B=8 -> one batch per NeuronCore.
"""

import numpy as np

import concourse.bass as bass
import concourse.tile as tile
from concourse import mybir
from concourse.bass_utils import run_bass_kernel_spmd

F16 = mybir.dt.float16
F32 = mybir.dt.float32

N_CORES = 8
NPTS = 4096          # points per core (batch dim sharded)
D = 256              # in channels
F = 256              # out channels
P = 128              # partitions
SN = 512             # points per full super-step
NS = NPTS // SN
HN = 2 * SN          # (d_half, n) packed axis
RSQRT2 = 0.7071067811865476

# (superstep, offset, count): half-size segments at head and tail
SEGS = ([(0, 0, 256), (0, 256, 256)]
        + [(s, 0, 512) for s in range(1, NS - 1)]
        + [(NS - 1, 0, 256), (NS - 1, 256, 256)])

ADD = mybir.AluOpType.add
SUB = mybir.AluOpType.subtract
MUL = mybir.AluOpType.mult


def _act_raw(nc, out, in_, func, scale=1.0):
    """InstActivation without the wrapper's Reciprocal guard (fp16 path)."""
    eng = nc.scalar
    ins = [eng.lower_ap(in_),
           mybir.ImmediateValue(dtype=mybir.dt.float32, value=0.0),
           mybir.ImmediateValue(dtype=mybir.dt.float32, value=scale),
           mybir.ImmediateValue(dtype=mybir.dt.float32, value=0.0)]
    return eng.add_instruction(mybir.InstActivation(
        name=nc.get_next_instruction_name(), func=func,
        ins=ins, outs=[eng.lower_ap(out)]))


def _split_multi_waits(nc):
    """This container's walrus rejects instructions carrying more than one
    sync wait. Hoist extra waits onto same-engine NoOps inserted directly
    before the offending instruction."""
    ctr = 0
    for f in nc.m.functions:
        for bb in f.blocks:
            out = []
            for inst in bb.instructions:
                si = inst.sync_info
                if si is not None and si.on_wait and len(si.on_wait) > 1:
                    waits = list(si.on_wait)
                    for w in waits[:-1]:
                        nop = mybir.InstNoOp(
                            name=f"waitnop_{ctr}", ins=[], outs=[])
                        ctr += 1
                        nop.engine = inst.engine
                        nop.bass_nofuse = True
                        nop.sync_info = mybir.SyncInfo(
                            on_wait=[w], on_update=[])
                        out.append(nop)
                    si.on_wait.clear()
                    si.on_wait.append(waits[-1])
                out.append(inst)
            bb.instructions[:] = out


def _dedupe_ldweights(nc):
    """Drop PE Ldweights reloads of an already-resident stationary (Tile
    emits one per matmul; the reload forces a full array drain)."""
    ctr = 0
    for f in nc.m.functions:
        for bb in f.blocks:
            out = []
            last_w = None
            for inst in bb.instructions:
                if inst.engine != mybir.EngineType.PE:
                    out.append(inst)
                    continue
                if inst.opcode == "Ldweights":
                    sig = str(inst.ins[0])
                    si = inst.sync_info
                    has_upd = bool(si is not None and si.on_update)
                    if sig == last_w and not has_upd:
                        if si is not None and si.on_wait:
                            nop = mybir.InstNoOp(
                                name=f"ldwnop_{ctr}", ins=[], outs=[])
                            ctr += 1
                            nop.engine = mybir.EngineType.PE
                            nop.bass_nofuse = True
                            nop.sync_info = mybir.SyncInfo(
                                on_wait=list(si.on_wait), on_update=[])
                            out.append(nop)
                        continue
                    last_w = sig
                elif inst.opcode != "Matmult":
                    last_w = None
                out.append(inst)
            bb.instructions[:] = out


def build_kernel(fixup=True):
    nc = bass.Bass("TRN2", target_bir_lowering=False, debug=False)
    # inputs, already fp16 + device-layout on host (pure relayout/cast)
    # J slots as [A, B, h, n] with A-major flat order (x, z, y, w):
    # jt[:, :, 0] = (x, y), jt[:, :, 1] = (z, w)
    j_d = nc.dram_tensor("JB", [NS, P, 2, 2, 2, SN], F16,
                         kind="ExternalInput").ap()
    x_d = nc.dram_tensor("XB", [NS, P, 3, 2, SN], F16,
                         kind="ExternalInput").ap()
    wa_d = nc.dram_tensor("WA", [P, 2, F], F16, kind="ExternalInput").ap()
    wb_d = nc.dram_tensor("WB", [P, 2, F], F16, kind="ExternalInput").ap()
    wbn_d = nc.dram_tensor("WBN", [P, 2, F], F16, kind="ExternalInput").ap()
    wc_d = nc.dram_tensor("WC", [P, 2, F], F16, kind="ExternalInput").ap()
    y_d = nc.dram_tensor("YB", [NS, P, 2, 3, SN], F16,
                         kind="ExternalOutput").ap()

    with tile.TileContext(nc) as tc:
        _body(nc, tc, j_d, x_d, (wa_d, wb_d, wbn_d, wc_d), y_d)
    from concourse import bass as _b
    _b._bass_rust.move_matmul_waits_to_ldweights(nc.m)
    _dedupe_ldweights(nc)
    if fixup:
        _split_multi_waits(nc)
    return nc


def _body(nc, tc, j_d, x_d, w_d, y_d):
    from contextlib import ExitStack
    ctx = ExitStack()
    with ctx:
        singles = ctx.enter_context(tc.tile_pool(name="singles", bufs=1))
        inp = ctx.enter_context(tc.tile_pool(name="inp", bufs=2))
        mid = ctx.enter_context(tc.tile_pool(name="mid", bufs=2))
        small = ctx.enter_context(tc.tile_pool(name="small", bufs=3))
        ysbp = ctx.enter_context(tc.tile_pool(name="ysb", bufs=2))
        ypps = ctx.enter_context(tc.tile_pool(name="ypps", bufs=1,
                                              space="PSUM"))

        # preload the reciprocal_and_small ACT table set while the first
        # DMAs are in flight (Square/Copy/Reciprocal all live in this set)
        warm = singles.tile([P, 16], F16, tag="actwarm")
        nc.gpsimd.memset(warm[:], 1.0)
        _act_raw(nc, warm[:], warm[:],
                 mybir.ActivationFunctionType.Reciprocal)

        # first segment's loads go out before the (PE-only) weights
        jt0, xt0 = _load_seg(nc, inp, j_d, x_d, SEGS[0])

        wts = []
        for name, wd in zip(("wa", "wb", "wbn", "wc"), w_d):
            wt = singles.tile([P, 2, F], F16, tag=f"w_{name}", name=f"w{name}")
            nc.gpsimd.dma_start(wt[:], wd)
            wts.append(wt)

        prev = None  # (yps, seg) pending evacuation
        loaded = (jt0, xt0)
        for gi, seg in enumerate(SEGS):
            jt, xt = loaded
            if gi + 1 < len(SEGS):
                loaded = _load_seg(nc, inp, j_d, x_d, SEGS[gi + 1])

            sn = seg[2]
            hn = 2 * sn
            # ---- ACT: half-squares; flat slots (xx, zz, yy, ww)/2 ----
            sq = mid.tile([P, 2, 2, HN], F16, tag="sq", name="sq")
            nc.scalar.activation(sq[:, :, :, :hn], jt[:, :, :, :hn],
                                 mybir.ActivationFunctionType.Square,
                                 scale=RSQRT2)

            # ---- DVE chain (11 instrs, 27 units) ----
            # P4[a,b] = (x,y)_a * (z,w)_b  ->  (xz, xw, yz, yw)
            p4 = mid.tile([P, 2, 2, HN], F16, tag="p4", name="p4")
            nc.vector.tensor_tensor(
                out=p4[:, :, :, :hn],
                in0=jt[:, 0, :, :hn].unsqueeze(2).broadcast_to([P, 2, 2, hn]),
                in1=jt[:, 1, :, :hn].unsqueeze(1).broadcast_to([P, 2, 2, hn]),
                op=MUL)
            # q12 = (xx+yy, zz+ww)/2
            q12 = small.tile([P, 2, HN], F16, tag="q12", name="q12")
            nc.vector.tensor_tensor(out=q12[:, :, :hn],
                                    in0=sq[:, :, 0, :hn],
                                    in1=sq[:, :, 1, :hn], op=ADD)
            sh = small.tile([P, HN], F16, tag="sh", name="sh")
            nc.vector.tensor_tensor(out=sh[:, :hn], in0=q12[:, 0, :hn],
                                    in1=q12[:, 1, :hn], op=ADD)
            # ACT: invh = 2/s (issued as early as possible)
            invh = small.tile([P, HN], F16, tag="invh", name="invh")
            _act_raw(nc, invh[:, :hn], sh[:, :hn],
                     mybir.ActivationFunctionType.Reciprocal)

            vt = mid.tile([P, 3, HN], F16, tag="vt", name="vt")
            nc.vector.tensor_tensor(out=vt[:, 2, :hn],
                                    in0=q12[:, 1, :hn],
                                    in1=q12[:, 0, :hn], op=SUB)
            nc.vector.tensor_tensor(out=vt[:, 0, :hn],
                                    in0=p4[:, 0, 0, :hn],
                                    in1=p4[:, 1, 1, :hn], op=ADD)
            nc.vector.tensor_tensor(out=vt[:, 1, :hn],
                                    in0=p4[:, 1, 0, :hn],
                                    in1=p4[:, 0, 1, :hn], op=SUB)
            g = mid.tile([P, 3, HN], F16, tag="g", name="g")
            nc.vector.tensor_tensor(
                out=g[:, :, :hn], in0=vt[:, :, :hn],
                in1=invh[:, :hn].unsqueeze(1).broadcast_to([P, 3, hn]),
                op=MUL)
            # O[a,b] = g_a * x_b : diag = pd, off-diag = cross pieces
            ot = mid.tile([P, 3, 3, HN], F16, tag="ot", name="ot")
            nc.vector.tensor_tensor(
                out=ot[:, :, :, :hn],
                in0=g[:, :, :hn].unsqueeze(2).broadcast_to([P, 3, 3, hn]),
                in1=xt[:, :, :hn].unsqueeze(1).broadcast_to([P, 3, 3, hn]),
                op=MUL)
            d1 = small.tile([P, HN], F16, tag="d1", name="d1")
            nc.vector.tensor_tensor(out=d1[:, :hn], in0=ot[:, 0, 0, :hn],
                                    in1=ot[:, 1, 1, :hn], op=ADD)
            dot = small.tile([P, HN], F16, tag="dot", name="dot")
            nc.vector.tensor_tensor(out=dot[:, :hn], in0=d1[:, :hn],
                                    in1=ot[:, 2, 2, :hn], op=ADD)
            ct = mid.tile([P, 3, HN], F16, tag="ct", name="ct")
            nc.vector.tensor_tensor(
                out=ct[:, :, :hn], in0=g[:, :, :hn],
                in1=dot[:, :hn].unsqueeze(1).broadcast_to([P, 3, hn]),
                op=MUL)

            # ---- PE: weights stationary, terms moving; ct-block last ----
            yp0 = ypps.tile([P, 3, SN], F32, tag="yp0", name="yp0")
            yp1 = ypps.tile([P, 3, SN], F32, tag="yp1", name="yp1")
            yps = (yp0, yp1)
            wa, wb, wbn, wc = wts
            crp = [(1, 2), (2, 0), (0, 1)]
            crn = [(2, 1), (0, 2), (1, 0)]
            pieces = [
                (wa, lambda i, h: xt[:, i, h * sn:(h + 1) * sn]),
                (wb, lambda i, h: ot[:, crp[i][0], crp[i][1],
                                     h * sn:(h + 1) * sn]),
                (wbn, lambda i, h: ot[:, crn[i][0], crn[i][1],
                                      h * sn:(h + 1) * sn]),
                (wc, lambda i, h: ct[:, i, h * sn:(h + 1) * sn]),
            ]
            for pi, (wt, term) in enumerate(pieces):
                for fh in range(2):
                    for h in range(2):
                        piece = wt[:, h, fh * P:(fh + 1) * P]
                        for i in range(3):
                            nc.tensor.matmul(
                                yps[fh][:, i, :sn],
                                lhsT=piece,
                                rhs=term(i, h),
                                start=(pi == 0 and h == 0),
                                stop=(pi == 3 and h == 1))

            # ---- evacuate the PREVIOUS segment's PSUM ----
            if prev is not None:
                _evac(nc, ysbp, prev, y_d)
            prev = (yps, seg)
        _evac(nc, ysbp, prev, y_d)


def _load_seg(nc, inp, j_d, x_d, seg):
    s, off, sn = seg
    hn = 2 * sn
    jt = inp.tile([P, 2, 2, HN], F16, tag="jt", name="jt")
    xt = inp.tile([P, 3, HN], F16, tag="xt", name="xt")
    if sn == SN:
        nc.sync.dma_start(jt[:, :, :, :hn],
                          j_d[s][:, :, :, :, off:off + sn])
        nc.sync.dma_start(xt[:, :, :hn], x_d[s][:, :, :, off:off + sn])
    else:
        for h in range(2):
            nc.sync.dma_start(jt[:, :, :, h * sn:(h + 1) * sn],
                              j_d[s][:, :, :, h, off:off + sn])
        for h in range(2):
            nc.sync.dma_start(xt[:, :, h * sn:(h + 1) * sn],
                              x_d[s][:, :, h, off:off + sn])
    return jt, xt


def _evac(nc, ysbp, prev, y_d):
    (yp0, yp1), (s, off, sn) = prev
    ysb = ysbp.tile([P, 2, 3, SN], F16, tag="ysb", name="ysb")
    nc.scalar.copy(ysb[:, 0, :, :sn], yp0[:, :, :sn])
    nc.scalar.copy(ysb[:, 1, :, :sn], yp1[:, :, :sn])
    nc.gpsimd.dma_start(y_d[s][:, :, :, off:off + sn], ysb[:, :, :, :sn])


_BUILT = {}

# test-harness hooks (ignored in normal use)
TRACE = False
LAST_EXEC_NS = None
LAST_RESULT = None


def _get_nc():
    if "nc" not in _BUILT:
        _BUILT["nc"] = build_kernel()
    return _BUILT["nc"]


def _pack_inputs(X, J, A, B, C):
    def packw(M):
        return np.ascontiguousarray(
            M.T.astype(np.float16).reshape(2, P, F).transpose(1, 0, 2))

    wa, wb, wbn, wc = packw(A), packw(B), packw(-B), packw(C - A)

    in_maps = []
    for b in range(N_CORES):
        # [n, d, c] -> [s, p, c, h, nn]
        xb = X[b].astype(np.float16).reshape(NS, SN, 2, P, 3)
        xb = np.ascontiguousarray(xb.transpose(0, 3, 4, 2, 1))
        # J slots natural (x, y, z, w): A-pairs (x,y)/(z,w) feed the
        # product outer; B-columns (x,z)/(y,w)... (xx,zz)+(yy,ww) feed q12
        jb = J[b].astype(np.float16).reshape(NS, SN, 2, P, 4)
        jb = np.ascontiguousarray(jb.transpose(0, 3, 4, 2, 1)).reshape(
            NS, P, 2, 2, 2, SN)
        in_maps.append({
            "XB": xb, "JB": jb,
            "WA": wa, "WB": wb, "WBN": wbn, "WC": wc,
        })
    return in_maps


def kernel(X, J, A, B, C):
    """X [8,4096,256,3] f32, J [8,4096,256,4] f32, A/B/C [256,256] f32
    -> Y [8,4096,256,3] f32."""
    X = np.asarray(X)
    J = np.asarray(J)
    A = np.asarray(A, dtype=np.float32)
    B = np.asarray(B, dtype=np.float32)
    C = np.asarray(C, dtype=np.float32)

    nc = _get_nc()
    in_maps = _pack_inputs(X, J, A, B, C)
    global LAST_EXEC_NS, LAST_RESULT
    res = run_bass_kernel_spmd(nc, in_maps, core_ids=list(range(N_CORES)),
                               trace=TRACE)
    LAST_EXEC_NS = res.exec_time_ns
    LAST_RESULT = res
    # device YB [s, p(f_local), fh, i, nn] -> [n, f, i] fp32
    out = np.empty((N_CORES, NPTS, F, 3), dtype=np.float32)
    for b in range(N_CORES):
        yb = res.results[b]["YB"].reshape(NS, P, 2, 3, SN).astype(np.float32)
        out[b] = yb.transpose(0, 4, 2, 1, 3).reshape(NPTS, F, 3)
    return np.ascontiguousarray(out)
